# revision 1
# baseline (speedup 1.0000x reference)
"""Trainium2 Bass kernel for nn_MultiHeadMixer.

Reference computation (B=4, S=2048, E=1024, H=16, D=64):
    xp = x @ inp_w.T + inp_b                      # (B,S,E)
    xh[b,h,d,s] = xp[b,s,h*D+d]
    y0[b,h,d,t] = sum_{s<=t} xh[b,h,d,s] * weight[h,t-s]   # causal Toeplitz
    y2 = y0 / cumsum(weight)[h,t] + bias[h,t]
    out[b,t,:] = (y2 reshaped to (E,)) @ out_w.T

Sharding (8 cores): core c = (batch b=c//2, head-group hg=c%2 of 8 heads,
embed cols 512*hg..512*hg+512).  Each core computes a full-(E) partial of
out[b].T in bf16; host sums the two head-group partials per batch and
transposes.

On-device layout: everything runs in the "transposed" domain [feature, seq]:
  proj1:  xp[s,c]   = sum_e xT[e,s] * w1T[e,c]           (PE, K=e)
  mixer:  y0[d,t]   = sum_s xp[s,d] * Toeplitz[s,t]      (PE, K=s, per head)
  proj2:  outT[e',t]= sum_c w2T[c,e'] * y2[c,t]          (PE, K=c)
The causal Toeplitz matmul uses per-head strip tiles Tp[p, kap] =
weight[h, kap-384-p] so every (s-block i, t-quad q) block of the Toeplitz
matrix is a contiguous 512-column slice of one SBUF tile.  Head pairs share
one PSUM tile (partitions 0-63 / 64-127 -> col-strip tiling on the PE), and
their matmuls are interleaved so the two strips execute concurrently.

Epilogue trick: each mixer chain starts with a K=2 PE matmul
(lhsT = half-indicator, rhs = compact biasn rows) that resets the PSUM
bank to biasn[h,t] = bias[h,t]*cumsum(weight)[h,t]; the Toeplitz matmuls
accumulate on top (start=False), and a single DVE multiply by 1/cumsum
yields y2 = y0/norm + bias directly in bf16.  Keeping the preload on the
PE makes the write/accumulate ordering trivially race-free.

Further structure: output partials are stored in bf16 (host sums in
f32); one unified 8-bank PSUM pool rotates across proj1/mixer/proj2;
pools live outside the timing loop; loads split across the SP ring
(xT + Toeplitz strips) and ACT ring (weights, constants, stores).
"""

import contextlib

import numpy as np
import ml_dtypes

import concourse.bass as bass
import concourse.bacc as bacc
import concourse.mybir as mybir
import concourse.tile as tile
from concourse.bass_types import AP
from concourse.bass_utils import run_bass_kernel_spmd

B, S, E, H = 4, 2048, 1024, 16
D = E // H
N_CORES = 8
HPC = 8          # heads per core
CPC = 512        # embed cols per core
SB = S // 128    # 16 s-blocks
EB = E // 128    # 8 e-blocks
TQ = S // 512    # 4 t-quads
TPW = 2176       # Toeplitz strip tile width: 2048 + 128 diagonal spill

BF16 = mybir.dt.bfloat16
F32 = mybir.dt.float32
NPBF16 = ml_dtypes.bfloat16

_CACHED = {}


def emit_body(nc, tc, aps, P):
    xT, w1T, b1x, w2T, Tps, invX, bn2, ind2, outT = aps
    (xt_pool, w1_pool, w2_pool, tp_pool, inv_pool, bn2_pool,
     ind2_pool, b1_pool, xp_pool, y2_pool, ost_pool, ps_pool) = P
    # ---- input loads, in consumption order, spread over 3 rings ----
    # sync ring: xT (the big stream).  scalar ring: everything else
    # (HWDGE rings exist only on SP and Activation).
    w1_t = []
    for k in range(EB):
        w = w1_pool.tile([128, CPC], BF16, tag="w1", name=f"w1_{k}")
        nc.scalar.dma_start(w[:], w1T[128 * k:128 * (k + 1), :])
        w1_t.append(w)
    b1_t = b1_pool.tile([128, CPC], F32, tag="b1", name="b1_t")
    nc.scalar.dma_start(b1_t[:], b1x[:])
    xt_t = []
    for k in range(EB):
        t = xt_pool.tile([128, S], BF16, tag="xt", name=f"xt_{k}")
        nc.sync.dma_start(t[:], xT[128 * k:128 * (k + 1), :])
        xt_t.append(t)

    tp_t = []
    for h in range(HPC):
        t = tp_pool.tile([128, 2048], BF16, tag="tp", name=f"tp_{h}")
        nc.scalar.dma_start(t[:], Tps[h])
        tp_t.append(t)
    w2_t = []
    for k in range(4):
        t = w2_pool.tile([128, E], BF16, tag="w2", name=f"w2_{k}")
        nc.scalar.dma_start(t[:], w2T[128 * k:128 * (k + 1), :])
        w2_t.append(t)

    # ---- constant panels ----
    # invn: host-broadcast f32 panels (DVE multiplicand); partitions
    # 0-63 = head 2hp, 64-127 = head 2hp+1 (matching mixer PSUM halves).
    # biasn: compact 2-row tile; ind2 selects the partition half so a
    # K=2 PE matmul materializes the biasn panel directly in PSUM.
    invn_t = []
    for hp in range(4):
        t = inv_pool.tile([128, S], BF16, tag="inv", name=f"invn_{hp}")
        nc.scalar.dma_start(t[:], invX[hp])
        invn_t.append(t)
    bn2_t = bn2_pool.tile([2, 4 * S], BF16, tag="bn2", name="bn2_t")
    nc.scalar.dma_start(bn2_t[:], bn2[:])
    ind2_t = ind2_pool.tile([2, 128], BF16, tag="ind2", name="ind2_t")
    nc.scalar.dma_start(ind2_t[:], ind2[:])

    # ---- proj1: xp[s-blk][128, 512] ----
    xp_t = []
    for m in range(SB):
        ps = ps_pool.tile([128, CPC], F32, tag="ps", name=f"ps1_{m}")
        for k in range(EB):
            nc.tensor.matmul(
                ps[:],
                xt_t[k][:, 128 * m:128 * (m + 1)],
                w1_t[k][:],
                start=(k == 0),
                stop=(k == EB - 1),
            )
        xp = xp_pool.tile([128, CPC], BF16, tag="xp", name=f"xp_{m}")
        nc.vector.tensor_add(xp[:], ps[:], b1_t[:])
        xp_t.append(xp)

    # ---- mixer: all quads, head pairs on PSUM partition halves ----
    y2_t = {}
    for q in range(TQ):
        n_i = 4 * q + 4
        tcol = slice(512 * q, 512 * (q + 1))
        for hp in range(4):
            ps = ps_pool.tile([128, CPC], F32, tag="ps",
                               name=f"psm_{hp}_{q}")
            # preload biasn = bias*norm via a K=2 PE matmul (start=True
            # resets the bank); the Toeplitz matmuls accumulate on top
            nc.tensor.matmul(
                ps[:],
                ind2_t[:],
                bn2_t[0:2, hp * S + 512 * q:hp * S + 512 * (q + 1)],
                start=True,
                stop=False,
                skip_group_check=True,
            )

            def mix_mm(par, i):
                h = 2 * hp + par
                prow = slice(64 * par, 64 * par + 64)
                off = 128 * (4 * q - i)
                # leading blocks with j<i are zero: skip them
                ncol0 = 128 * (i - 4 * q) if i > 4 * q else 0
                nc.tensor.matmul(
                    ps[prow, ncol0:CPC],
                    xp_t[i][:, 64 * h:64 * (h + 1)],
                    tp_t[h][:, off + ncol0:off + CPC],
                    start=False,
                    stop=(i == n_i - 1),
                    skip_group_check=True,
                )

            for i in range(n_i):
                mix_mm(0, i)
                mix_mm(1, i)

            y2 = y2_pool.tile([128, CPC], BF16, tag="y2",
                              name=f"y2_{hp}_{q}")
            nc.vector.tensor_mul(y2[:], ps[:], invn_t[hp][:, tcol])
            y2_t[(hp, q)] = y2

    # ---- proj2: all quads ----
    for q in range(TQ):
        for n in range(EB):
            ps = ps_pool.tile([128, CPC], F32, tag="ps",
                               name=f"ps2_{n}_{q}")
            for k in range(4):
                nc.tensor.matmul(
                    ps[:],
                    w2_t[k][:, 128 * n:128 * (n + 1)],
                    y2_t[(k, q)][:],
                    start=(k == 0),
                    stop=(k == 3),
                )
            ost = ost_pool.tile([128, CPC], BF16, tag="ost",
                                name=f"ost_{n}_{q}")
            # drain on scalar/vector alternately to balance engines
            if n % 2 == 0:
                nc.scalar.copy(ost[:], ps[:])
            else:
                nc.vector.tensor_scalar_add(ost[:], ps[:], 0.0)
            nc.scalar.dma_start(
                outT[128 * n:128 * (n + 1), 512 * q:512 * (q + 1)],
                ost[:],
            )


def build_program(loop_n=None):
    nc = bacc.Bacc("TRN2", target_bir_lowering=False, debug=False,
                   num_devices=N_CORES)

    aps = (
        nc.dram_tensor("xT", [E, S], BF16, kind="ExternalInput").ap(),
        nc.dram_tensor("w1T", [E, CPC], BF16, kind="ExternalInput").ap(),
        nc.dram_tensor("b1x", [128, CPC], F32, kind="ExternalInput").ap(),
        nc.dram_tensor("w2T", [CPC, E], BF16, kind="ExternalInput").ap(),
        nc.dram_tensor("Tps", [HPC, 128, 2048], BF16, kind="ExternalInput").ap(),
        nc.dram_tensor("invX", [4, 128, S], BF16, kind="ExternalInput").ap(),
        nc.dram_tensor("bn2", [2, 4 * S], BF16, kind="ExternalInput").ap(),
        nc.dram_tensor("ind2", [2, 128], BF16, kind="ExternalInput").ap(),
        nc.dram_tensor("outT", [E, S], BF16, kind="ExternalOutput").ap(),
    )

    with tile.TileContext(nc) as tc:
        with (
            tc.tile_pool(name="xt", bufs=EB) as xt_pool,
            tc.tile_pool(name="w1", bufs=EB) as w1_pool,
            tc.tile_pool(name="w2", bufs=4) as w2_pool,
            tc.tile_pool(name="tp", bufs=HPC) as tp_pool,
            tc.tile_pool(name="inv", bufs=4) as inv_pool,
            tc.tile_pool(name="bn2", bufs=1) as bn2_pool,
            tc.tile_pool(name="ind2", bufs=1) as ind2_pool,
            tc.tile_pool(name="b1", bufs=1) as b1_pool,
            tc.tile_pool(name="xp", bufs=SB) as xp_pool,
            tc.tile_pool(name="y2", bufs=16) as y2_pool,
            tc.tile_pool(name="ost", bufs=4) as ost_pool,
            tc.tile_pool(name="ps", bufs=8, space="PSUM") as ps_pool,
        ):
            P = (xt_pool, w1_pool, w2_pool, tp_pool, inv_pool, bn2_pool,
                 ind2_pool, b1_pool, xp_pool, y2_pool, ost_pool, ps_pool)
            with (tc.For_i(0, loop_n, 1) if loop_n else contextlib.nullcontext()):
                emit_body(nc, tc, aps, P)

    nc.compile()
    return nc


def host_prep(x, weight, bias, inp_w, inp_b, out_w):
    """Build the 8 per-core input maps (host-side shard + layout prep)."""
    x = np.asarray(x, np.float32)
    weight = np.asarray(weight, np.float32)
    bias = np.asarray(bias, np.float32)
    inp_w = np.asarray(inp_w, np.float32)
    inp_b = np.asarray(inp_b, np.float32)
    out_w = np.asarray(out_w, np.float32)

    norm = np.cumsum(weight, axis=1)            # (H, S)
    invn = 1.0 / norm
    biasn = bias * norm

    xT_b = [np.ascontiguousarray(x[b].T).astype(NPBF16) for b in range(B)]

    hg_pack = []
    for hg in range(2):
        heads = range(HPC * hg, HPC * hg + HPC)
        cols = slice(CPC * hg, CPC * hg + CPC)
        w1T = np.ascontiguousarray(inp_w[cols, :].T).astype(NPBF16)
        b1x = np.broadcast_to(inp_b[cols], (128, CPC)).astype(np.float32).copy()
        w2T = np.ascontiguousarray(out_w[:, cols].T).astype(NPBF16)
        Tps = np.zeros((HPC, 128, 2048), NPBF16)
        for hi, h in enumerate(heads):
            wrow = weight[h]
            for p in range(128):
                Tps[hi, p, p:2048] = wrow[:2048 - p]
        invX = np.zeros((4, 128, S), NPBF16)
        bn2 = np.zeros((2, 4 * S), NPBF16)
        for hp in range(4):
            h0 = HPC * hg + 2 * hp
            invX[hp, :64] = invn[h0]
            invX[hp, 64:] = invn[h0 + 1]
            bn2[0, hp * S:(hp + 1) * S] = biasn[h0]
            bn2[1, hp * S:(hp + 1) * S] = biasn[h0 + 1]
        ind2 = np.zeros((2, 128), NPBF16)
        ind2[0, 0:64] = 1
        ind2[1, 64:128] = 1
        hg_pack.append(dict(w1T=w1T, b1x=b1x, w2T=w2T, Tps=Tps,
                            invX=invX, bn2=bn2, ind2=ind2))

    in_maps = []
    for c in range(N_CORES):
        b, hg = c // 2, c % 2
        m = dict(hg_pack[hg])
        m["xT"] = xT_b[b]
        in_maps.append(m)
    return in_maps


def kernel(x, weight, bias, inp_w, inp_b, out_w):
    if "nc" not in _CACHED:
        _CACHED["nc"] = build_program()
    nc = _CACHED["nc"]

    in_maps = host_prep(x, weight, bias, inp_w, inp_b, out_w)
    res = run_bass_kernel_spmd(nc, in_maps, core_ids=list(range(N_CORES)))

    out = np.empty((B, S, E), np.float32)
    for b in range(B):
        p0 = np.asarray(res.results[2 * b]["outT"], dtype=np.float32)
        p1 = np.asarray(res.results[2 * b + 1]["outT"], dtype=np.float32)
        out[b] = (p0 + p1).T
    return out

